# revision 2
# baseline (speedup 1.0000x reference)
"""HSTU block kernel v2 for 8 TRN2 NeuronCores (Bass/Tile, fp32r matmuls).

Sharding: phase 1 (f1 + attention + u-gating) is data-parallel over batch
(B=2) x tensor-parallel over head groups (4 heads/core = 2 pairs). Phase 2
(ln1 -> f2 -> +residual -> ln2) is row-parallel (512 rows/core), with ln1
folded analytically into the f2 weights so no normalization pass is needed
before the matmul.

Key layout: heads are processed in PAIRS occupying SBUF partitions [0:64] /
[64:128]; QK/AV matmuls for the two heads auto-derive row/col tile_position
(64-row / 64-col groups) and run concurrently in the PE array on HW.
Attention score tiles for a pair live in one 2-bank [128,1024] PSUM tile so
bias-add / mask / silu process both heads in one instruction. The relative
position bias adds run on DVE + GpSimd(Pool), load-balanced; the PE never
touches them.
"""
import os
import numpy as np

import concourse.bacc as bacc
import concourse.mybir as mybir
from concourse import library_config
from concourse.tile import TileContext
from concourse.bass_utils import run_bass_kernel_spmd

fp32 = mybir.dt.float32
fp32r = mybir.dt.float32r
bf16 = mybir.dt.bfloat16
AF = mybir.ActivationFunctionType
ALU = mybir.AluOpType

B, S, D, H, M = 2, 2048, 1024, 16, 4096
HD = D // H          # 64
EPS = 1e-5
P = 128
NB = S // P          # 16 seq blocks of 128
NG = S // 512        # 4 q-groups of 512
DC = 4 * HD          # 256 features per core in phase 1

_CACHE = {}


class Balancer:
    """Greedy least-loaded assignment of elementwise ops to DVE / Pool.

    GPSIMD (Pool) has no PSUM port, so ops touching PSUM are DVE-only.
    """

    def __init__(self, nc, dve_bias=0.0, pool_bias=0.0):
        self.nc = nc
        self.load = {"dve": dve_bias, "pool": pool_bias}

    def pick(self, width, psum=False):
        # cost estimates per engine for a [128, width]-elem op (ns)
        cd = width * 1.0417 + 250
        cp = width * 0.833 / 0.42 + 131
        if psum or self.load["dve"] + cd <= self.load["pool"] + cp:
            self.load["dve"] += cd
            return self.nc.vector
        self.load["pool"] += cp
        return self.nc.gpsimd


# ---------------------------------------------------------------- kernel A
def build_kernel_a(plan, jmin, n_strip, n_partial):
    """plan[g] = [(kt, lead, trail, cls4)], strip holds tiles jmin..jmin+n_strip-1."""
    nc = bacc.Bacc("TRN2", target_bir_lowering=False, debug=False, num_devices=8)

    xT = nc.dram_tensor("xT", [D, S], bf16, kind="ExternalInput")
    w1T_qku = nc.dram_tensor("w1T_qku", [D, 768], bf16, kind="ExternalInput")
    w1T_v = nc.dram_tensor("w1T_v", [D, DC], bf16, kind="ExternalInput")
    b1_2d = nc.dram_tensor("b1_2d", [P, 6], fp32, kind="ExternalInput")
    b1v_bc2 = nc.dram_tensor("b1v_bc2", [P, 512], bf16, kind="ExternalInput")
    inv128 = nc.dram_tensor("inv128", [P, P], bf16, kind="ExternalInput")
    strip = nc.dram_tensor("strip", [P, n_strip * P], fp32, kind="ExternalInput")
    parts = nc.dram_tensor("parts", [max(n_partial, 1), P, P], fp32,
                           kind="ExternalInput")
    yT_out = nc.dram_tensor("yT_out", [DC, S], bf16, kind="ExternalOutput")

    with TileContext(nc) as tc:
        with tc.tile_pool(name="const", bufs=1) as cpool, \
             tc.tile_pool(name="wpool", bufs=2) as wpool, \
             tc.tile_pool(name="big", bufs=1) as big, \
             tc.tile_pool(name="att", bufs=4) as apool, \
             tc.tile_pool(name="out", bufs=3) as opool, \
             tc.tile_pool(name="ps", bufs=1, space="PSUM") as ps:

            bal = Balancer(nc)

            # ---- critical-path DMAs: w1 qku interleaved with slab-0 x ----
            # tiny b1 first (gates the first silu); ACT silu-table prewarm
            b1_sb = cpool.tile([P, 6], fp32, name="b1_sb")
            nc.sync.dma_start(b1_sb[:], b1_2d[:])
            warm = cpool.tile([1, 1], fp32, name="warm")
            nc.gpsimd.memset(warm[:], 0.0)
            warm2 = cpool.tile([1, 1], fp32, name="warm2")
            nc.scalar.activation(warm2[:], warm[:], AF.Silu, scale=1.0)
            wq = []
            xs0 = big.tile([P, 8 * 512], bf16, name="xs0", tag="xs", bufs=2)
            xv = xT.rearrange("(c p) s -> p c s", p=P)
            for k in range(8):
                t = big.tile([P, 768], bf16, name=f"wq{k}", tag=f"wq{k}")
                nc.sync.dma_start(t[:], w1T_qku[k * P:(k + 1) * P, :])
                wq.append(t)
                if k % 2 == 1:
                    c = k // 2
                    nc.sync.dma_start(
                        xs0.rearrange("p (c s) -> p c s", c=8)[:, 2 * c:2 * c + 2,
                                                              0:512],
                        xv[:, 2 * c:2 * c + 2, 0:512])
            inv128_sb = cpool.tile([P, P], bf16, name="inv128_sb")
            nc.sync.dma_start(inv128_sb[:], inv128[:])
            b1v_sb = cpool.tile([P, 512], bf16, name="b1v_sb")
            nc.sync.dma_start(b1v_sb[:], b1v_bc2[:])
            parts_sb = []
            for i in range(n_partial):
                t = cpool.tile([P, P], fp32, name=f"part{i}", tag=f"part{i}")
                nc.sync.dma_start(t[:], parts[i])
                parts_sb.append(t)
            strip_sb = big.tile([P, n_strip * P], fp32, name="strip_sb",
                                tag="strip")
            nc.sync.dma_start(strip_sb[:], strip[:])
            wv = big.tile([P, 8 * DC], bf16, name="wv", tag="wv")
            nc.sync.dma_start(
                wv.rearrange("p (c f) -> p c f", c=8),
                w1T_v.rearrange("(c p) f -> p c f", p=P))

            # persistent k (pairs) and v
            kh = [big.tile([P, S], bf16, name=f"khp{p}", tag=f"khp{p}")
                  for p in range(2)]
            v_sb = big.tile([P, NB * DC], bf16, name="v_sb", tag="v_sb")
            # K=1 zero operands: one start=True matmul initializes a full
            # avp bank so the col-tiled AV pair can pure-accumulate into it
            zrow = cpool.tile([1, 640], bf16, name="zrow")
            nc.gpsimd.memset(zrow[:], 0.0)

            # ---------------- f1: qku + v, feature-major / natural --------
            def f1_slab(sg):
                sl = slice(sg * 512, (sg + 1) * 512)
                if sg == 0:
                    xs = xs0
                else:
                    xs = big.tile([P, 8 * 512], bf16, name="xs", tag="xs",
                                  bufs=2)
                    nc.sync.dma_start(
                        xs.rearrange("p (c s) -> p c s", c=8),
                        xv[:, :, sl])
                qq = [wpool.tile([P, 512], bf16, name="qq", tag=f"qq{p}",
                                 bufs=2) for p in range(2)]
                uu = [wpool.tile([P, 512], fp32, name="uu", tag=f"uu{p}",
                                 bufs=2) for p in range(2)]
                for fc in range(6):
                    pt = ps.tile([P, 512], fp32, name="f1ps", tag="f1", bufs=2)
                    for k in range(8):
                        nc.tensor.matmul(pt[:], wq[k][:, fc * P:(fc + 1) * P],
                                         xs[:, k * 512:(k + 1) * 512],
                                         start=(k == 0), stop=(k == 7))
                    bias = b1_sb[:, fc:fc + 1]
                    if fc < 2:
                        nc.scalar.activation(qq[fc][:], pt[:], AF.Silu,
                                             bias=bias, scale=1.0)
                    elif fc < 4:
                        nc.scalar.activation(kh[fc - 2][:, sl], pt[:], AF.Silu,
                                             bias=bias, scale=1.0)
                    else:
                        nc.scalar.activation(uu[fc - 4][:], pt[:], AF.Silu,
                                             bias=bias, scale=1.0)
                # v: two chains of 2 seq-blocks each (512 tokens -> 4 blocks)
                for scp in range(2):
                    pt = ps.tile([P, 512], fp32, name="fvps", tag="f1", bufs=2)
                    for k in range(8):
                        for sc2 in range(2):
                            tok = (2 * scp + sc2) * P
                            first_mm = (k == 0 and sc2 == 0)
                            nc.tensor.matmul(
                                pt[:, sc2 * DC:(sc2 + 1) * DC],
                                xs[:, k * 512 + tok:k * 512 + tok + P],
                                wv[:, k * DC:(k + 1) * DC],
                                start=first_mm, stop=False)
                    nc.tensor.matmul(pt[:], inv128_sb[:], b1v_sb[:],
                                     start=False, stop=True)
                    base = (4 * sg + 2 * scp) * DC
                    nc.scalar.activation(v_sb[:, base:base + 512], pt[:],
                                         AF.Silu, scale=1.0)
                return qq, uu

            # ---------------- attention (per head pair) ----------------
            def attn_pair(p, g, qq, uu):
                kts = plan[g]
                if not kts:
                    return
                last_kt = kts[-1][0]
                full_first = (kts[0][1] == 0 and kts[0][2] == 0)
                avps = []
                for hp in range(2):
                    t = ps.tile([64, 512], fp32, name="avp", tag="avp", bufs=2)
                    if not full_first:
                        nc.tensor.matmul(t[:], zrow[:, 0:64],
                                         zrow[:, P:P + 512],
                                         start=True, stop=False)
                    avps.append(t)
                for ki, (kt, lead, trail, cls4) in enumerate(kts):
                    off, end = lead * P, 512 - trail * P
                    w = end - off
                    spp = ps.tile([P, 1024], fp32, name="spp", tag="spp",
                                  bufs=2)
                    for hp in range(2):
                        hs = slice(hp * 64, hp * 64 + 64)
                        nc.tensor.matmul(
                            spp[:, hp * 512 + off:hp * 512 + end],
                            kh[p][hs, kt * P:(kt + 1) * P],
                            qq[p][hs, off:end],
                            start=True, stop=True)
                    sb0 = (4 * g - kt + 15 - jmin) * P
                    att_in = apool.tile([P, 1024], fp32, name="att_in",
                                        tag="att_in", bufs=3)
                    sv = spp.rearrange("a (two w) -> a two w", two=2)
                    ai = att_in.rearrange("a (two w) -> a two w", two=2)
                    st = strip_sb[:, sb0 + off:sb0 + end].unsqueeze(1) \
                        .broadcast_to((P, 2, w))
                    nc.vector.tensor_tensor(ai[:, :, off:end],
                                            sv[:, :, off:end], st, ALU.add)
                    for j in range(lead, 4 - trail):
                        c = cls4[j]
                        if c >= 2:
                            aj = ai[:, :, j * P:(j + 1) * P]
                            pj = parts_sb[c - 2][:].unsqueeze(1) \
                                .broadcast_to((P, 2, P))
                            nc.gpsimd.tensor_tensor(aj, aj, pj, ALU.mult)
                    att = apool.tile([P, 1024], bf16, name="att", tag="att",
                                     bufs=4)
                    av = att.rearrange("a (two w) -> a two w", two=2)
                    nc.scalar.activation(av[:, :, off:end], ai[:, :, off:end],
                                         AF.Silu, scale=1.0)
                    vb = kt * DC + p * P
                    for hp in range(2):
                        nc.tensor.matmul(
                            avps[hp][:, off:end],
                            v_sb[:, vb + hp * 64:vb + hp * 64 + 64],
                            att[:, hp * 512 + off:hp * 512 + end],
                            start=(full_first and ki == 0),
                            stop=(kt == last_kt))
                yg = opool.tile([P, 512], bf16, name="yg", tag="yg", bufs=2)
                for hp in range(2):
                    nc.vector.tensor_tensor(yg[hp * 64:hp * 64 + 64, :],
                                            avps[hp][:],
                                            uu[p][hp * 64:hp * 64 + 64, :],
                                            ALU.mult)
                nc.sync.dma_start(
                    yT_out[p * P:(p + 1) * P, g * 512:(g + 1) * 512], yg[:])

            for sg in range(NG):
                qq, uu = f1_slab(sg)
                attn_pair(0, sg, qq, uu)
                attn_pair(1, sg, qq, uu)
    return nc


# ---------------------------------------------------------------- kernel B
def build_kernel_b():
    nc = bacc.Bacc("TRN2", target_bir_lowering=False, debug=False, num_devices=8)

    yT = nc.dram_tensor("yT", [D, 512], bf16, kind="ExternalInput")
    xTs = nc.dram_tensor("xTs", [D, 512], bf16, kind="ExternalInput")
    w2gT = nc.dram_tensor("w2gT", [D, D], bf16, kind="ExternalInput")
    cvec = nc.dram_tensor("cvec", [1, 2 * D], bf16, kind="ExternalInput")
    g2b2 = nc.dram_tensor("g2b2", [P, 16], fp32, kind="ExternalInput")
    onesp = nc.dram_tensor("onesp", [P, 1], fp32, kind="ExternalInput")
    monesp = nc.dram_tensor("monesp", [P, 1], bf16, kind="ExternalInput")
    monespf = nc.dram_tensor("monespf", [P, 1], fp32, kind="ExternalInput")
    ones512 = nc.dram_tensor("ones512", [1, 512], bf16, kind="ExternalInput")
    ident = nc.dram_tensor("ident", [P, P], bf16, kind="ExternalInput")
    outT = nc.dram_tensor("outT", [D, 512], fp32, kind="ExternalOutput")

    yv = yT.rearrange("(c p) s -> p c s", p=P)
    xv = xTs.rearrange("(c p) s -> p c s", p=P)
    wv = w2gT.rearrange("(c p) f -> p c f", p=P)

    with TileContext(nc) as tc:
        with tc.tile_pool(name="const", bufs=1) as cpool, \
             tc.tile_pool(name="big", bufs=1) as big, \
             tc.tile_pool(name="tmp", bufs=3) as tp, \
             tc.tile_pool(name="ps", bufs=1, space="PSUM") as ps:

            bal = Balancer(nc)

            nc.gpsimd.load_library(library_config.proxy)
            eps_sb = cpool.tile([1, 1], fp32, name="eps_sb")
            nc.gpsimd.memset(eps_sb[:], EPS)
            warm = cpool.tile([1, 1], fp32, name="warm")
            nc.scalar.activation(warm[:], eps_sb[:], AF.Sqrt, scale=1.0)

            # DMA order tuned for the dependency graph: first weight chunk
            # (starts f2 chain 0), tiny consts, y (stats + f2 rhs), xts +
            # cvec (gate the rank-1 chains), then remaining weight chunks
            # which pace the later f2 accumulation chains.
            yt = big.tile([P, 8 * 512], bf16, name="yt", tag="yt")
            w2g = big.tile([P, 8 * D], bf16, name="w2g", tag="w2g")
            xts = big.tile([P, 8 * 512], bf16, name="xts", tag="xts")
            onesp_sb = cpool.tile([P, 1], fp32r, name="onesp_sb")
            nc.sync.dma_start(onesp_sb[:], onesp[:].bitcast(fp32r))
            monesp_sb = cpool.tile([P, 1], bf16, name="monesp_sb")
            nc.sync.dma_start(monesp_sb[:], monesp[:])
            monespf_sb = cpool.tile([P, 1], fp32r, name="monespf_sb")
            nc.sync.dma_start(monespf_sb[:], monespf[:].bitcast(fp32r))
            for c in range(4):
                nc.sync.dma_start(
                    yt.rearrange("p (c s) -> p c s", c=8)
                    [:, 2 * c:2 * c + 2, :],
                    yv[:, 2 * c:2 * c + 2, :])
            ones512_sb = cpool.tile([1, 512], bf16, name="ones512_sb")
            nc.sync.dma_start(ones512_sb[:], ones512[:])
            cvec_sb = cpool.tile([1, 2 * D], bf16, name="cvec_sb")
            nc.sync.dma_start(cvec_sb[:], cvec[:])
            g2b2_sb = cpool.tile([P, 16], fp32, name="g2b2_sb")
            nc.sync.dma_start(g2b2_sb[:], g2b2[:])
            ident_sb = cpool.tile([P, P], bf16, name="ident_sb")
            nc.sync.dma_start(ident_sb[:], ident[:])
            xvv = xts.rearrange("p (c s) -> p c s", c=8)
            for k in range(8):
                nc.sync.dma_start(
                    w2g[:, k * D:(k + 1) * D],
                    wv[:, k, :])
                if k < 4:
                    nc.sync.dma_start(xvv[:, 2 * k:2 * k + 2, :],
                                      xv[:, 2 * k:2 * k + 2, :])

            def stats(src_slices, tag, mones):
                """accumulate -mean and +mean-of-squares; [1,512] PSUM."""
                n = len(src_slices)
                ss = ps.tile([1, 512], fp32, name=f"ss{tag}", tag="st", bufs=3)
                sq = ps.tile([1, 512], fp32, name=f"sq{tag}", tag="st", bufs=3)
                for k, ch in enumerate(src_slices):
                    nc.tensor.matmul(ss[:], mones[:], ch,
                                     start=(k == 0), stop=(k == n - 1))
                    sqt = tp.tile([P, 512], fp32r, name="sqt", tag="sqt",
                                  bufs=2)
                    nc.scalar.activation(sqt[:], ch, AF.Square,
                                         scale=1.0)
                    nc.tensor.matmul(sq[:], onesp_sb[:], sqt[:],
                                     start=(k == 0), stop=(k == n - 1))
                return ss, sq

            def norm_coefs(nmu, ms, tag):
                """nmu=-mean, ms=E[x^2] -> (rstd, s=-mu*rstd) [1,512] fp32r."""
                m2 = tp.tile([1, 512], fp32, name=f"m2{tag}", tag="vec", bufs=4)
                nc.scalar.activation(m2[:], nmu[:], AF.Square, scale=1.0)
                var = tp.tile([1, 512], fp32, name=f"var{tag}", tag="vec",
                              bufs=4)
                nc.vector.tensor_tensor(var[:], ms[:], m2[:], ALU.subtract)
                sd = tp.tile([1, 512], fp32, name=f"sd{tag}", tag="vec", bufs=4)
                nc.scalar.activation(sd[:], var[:], AF.Sqrt, bias=eps_sb[:],
                                     scale=1.0)
                rstd = tp.tile([1, 512], fp32r, name=f"rstd{tag}", tag="vecr",
                               bufs=4)
                nc.vector.reciprocal(rstd[:].bitcast(fp32), sd[:])
                s = tp.tile([1, 512], bf16, name=f"s{tag}", tag="vecr",
                             bufs=4)
                nc.vector.tensor_tensor(s[:], nmu[:],
                                        rstd[:].bitcast(fp32), ALU.mult)
                return rstd, s

            # ---- ln1 (folded): stats on raw y run concurrently with f2 ----
            ytc = [yt[:, k * 512:(k + 1) * 512] for k in range(8)]
            ss1, sq1 = stats(ytc, "a", monesp_sb)
            rstd1, s1 = norm_coefs(ss1, sq1, "a")
            pa_sb = tp.tile([P, 512], fp32, name="pa_sb", tag="bcs", bufs=3)
            nc.gpsimd.partition_broadcast(pa_sb[:], rstd1[:].bitcast(fp32))

            # ---- f2 on RAW y; post-scale z; rank-1 + residual; ln2 ----
            # processed in token halves so ln2 stats + apply of half 0
            # overlap the f2 accumulation chains of half 1
            y2 = big.tile([P, 8 * 512], bf16, name="y2", tag="y2")
            outl = big.tile([P, 8 * 512], fp32, name="outl", tag="outl")
            ov = outT.rearrange("(c p) s -> p c s", p=P)
            W = 256

            def norm_coefs_h(nmu, ms, tag):
                m2 = tp.tile([1, W], fp32, name=f"m2{tag}", tag="vech", bufs=4)
                nc.scalar.activation(m2[:], nmu[:], AF.Square, scale=1.0)
                var = tp.tile([1, W], fp32, name=f"var{tag}", tag="vech",
                              bufs=4)
                nc.vector.tensor_tensor(var[:], ms[:], m2[:], ALU.subtract)
                sd = tp.tile([1, W], fp32, name=f"sd{tag}", tag="vech", bufs=4)
                nc.scalar.activation(sd[:], var[:], AF.Sqrt, bias=eps_sb[:],
                                     scale=1.0)
                rstd = tp.tile([1, W], fp32, name=f"rstd{tag}", tag="vech",
                               bufs=4)
                nc.vector.reciprocal(rstd[:], sd[:])
                sh = tp.tile([1, W], fp32, name=f"sh{tag}", tag="vech", bufs=4)
                nc.vector.tensor_tensor(sh[:], nmu[:], rstd[:], ALU.mult)
                return rstd, sh

            # prebuild rank-1+residual tiles in SBUF (PE+ACT, overlaps
            # the w2g DMA window) so the endgame add can run on Pool
            rr_sb = {}
            for h in range(2):
                t0 = h * W
                for fc in range(8):
                    rr = ps.tile([P, W], fp32, name=f"rr{fc}", tag="rr",
                                 bufs=2)
                    nc.tensor.matmul(rr[:], ident_sb[:],
                                     xts[:, fc * 512 + t0:fc * 512 + t0 + W],
                                     start=True, stop=False)
                    nc.tensor.matmul(rr[:],
                                     cvec_sb[:, D + fc * P:D + (fc + 1) * P],
                                     ones512_sb[:, 0:W],
                                     start=False, stop=False)
                    nc.tensor.matmul(rr[:], cvec_sb[:, fc * P:(fc + 1) * P],
                                     s1[:, t0:t0 + W], start=False, stop=True)
                    t = big.tile([P, W], fp32, name=f"rrs{h}_{fc}",
                                 tag=f"rrs{h}_{fc}")
                    nc.scalar.activation(t[:], rr[:], AF.Copy, scale=1.0)
                    rr_sb[(h, fc)] = t

            for h in range(2):
                t0 = h * W
                ss2 = ps.tile([1, W], fp32, name=f"ss2{h}", tag="st", bufs=3)
                sq2 = ps.tile([1, W], fp32, name=f"sq2{h}", tag="st", bufs=3)
                for fc in range(8):
                    pt = ps.tile([P, W], fp32, name=f"f2ps{fc}", tag="f2",
                                 bufs=3)
                    for k in range(8):
                        nc.tensor.matmul(
                            pt[:],
                            w2g[:, k * D + fc * P:k * D + (fc + 1) * P],
                            yt[:, k * 512 + t0:k * 512 + t0 + W],
                            start=(k == 0), stop=(k == 7))
                    ch = y2[:, fc * 512 + t0:fc * 512 + t0 + W]
                    nc.vector.tensor_tensor(ch, pt[:],
                                            pa_sb[:, t0:t0 + W], ALU.mult)
                    nc.gpsimd.tensor_tensor(ch, ch, rr_sb[(h, fc)][:],
                                            ALU.add)
                    nc.tensor.matmul(ss2[:], monesp_sb[:], ch,
                                     start=(fc == 0), stop=(fc == 7))
                    sqt = tp.tile([P, W], fp32r, name="sqt2", tag="sqt2",
                                  bufs=2)
                    nc.scalar.activation(sqt[:], ch, AF.Square,
                                         scale=1.0)
                    nc.tensor.matmul(sq2[:], onesp_sb[:], sqt[:],
                                     start=(fc == 0), stop=(fc == 7))

                rstd2, s2 = norm_coefs_h(ss2, sq2, f"b{h}")
                pa2_sb = tp.tile([P, W], fp32, name="pa2_sb", tag="bcs",
                                 bufs=3)
                nc.gpsimd.partition_broadcast(pa2_sb[:], rstd2[:])
                pb2_sb = tp.tile([P, W], fp32, name="pb2_sb", tag="bcs",
                                 bufs=3)
                nc.gpsimd.partition_broadcast(pb2_sb[:], s2[:])
                for fc in range(8):
                    yc = y2[:, fc * 512 + t0:fc * 512 + t0 + W]
                    u1 = tp.tile([P, W], fp32, name="u1", tag="u1", bufs=4)
                    eng = bal.pick(W)
                    eng.tensor_tensor(u1[:], yc, pa2_sb[:], ALU.mult)
                    eng = bal.pick(W)
                    eng.tensor_tensor(u1[:], u1[:], pb2_sb[:], ALU.add)
                    oc = outl[:, fc * 512 + t0:fc * 512 + t0 + W]
                    eng = bal.pick(W)
                    eng.tensor_scalar(oc, u1[:],
                                      g2b2_sb[:, fc:fc + 1],
                                      g2b2_sb[:, 8 + fc:8 + fc + 1],
                                      ALU.mult, ALU.add)
                    if fc % 4 == 3:
                        nc.sync.dma_start(
                            ov[:, fc - 3:fc + 1, t0:t0 + W],
                            outl.rearrange("p (c s) -> p c s", c=8)
                            [:, fc - 3:fc + 1, t0:t0 + W])
    return nc


# ---------------------------------------------------------------- host side
def _classify_mask(mask):
    keep = (mask.reshape(S, S) >= 0)
    block_cls = [[0] * NB for _ in range(NB)]  # [kt][qb]
    partials = []
    pmap = {}
    for kt in range(NB):
        for qb in range(NB):
            sub = keep[qb * P:(qb + 1) * P, kt * P:(kt + 1) * P]
            if sub.all():
                block_cls[kt][qb] = 1
            elif not sub.any():
                block_cls[kt][qb] = 0
            else:
                tile = np.ascontiguousarray(sub.T.astype(np.float32))
                key = tile.tobytes()
                if key not in pmap:
                    pmap[key] = len(partials)
                    partials.append(tile)
                block_cls[kt][qb] = 2 + pmap[key]
    return block_cls, partials


def _plan_attn(block_cls):
    """Per (g, kt): lead/trail skip blocks + strip tile range used."""
    plan = {}
    jmin, jmax = 31, 0
    for g in range(NG):
        kts = []
        for kt in range(NB):
            cls4 = [block_cls[kt][4 * g + j] for j in range(4)]
            if all(c == 0 for c in cls4):
                continue
            lead = 0
            while cls4[lead] == 0:
                lead += 1
            trail = 0
            while cls4[3 - trail] == 0:
                trail += 1
            jj0 = 4 * g - kt + 15
            jmin = min(jmin, jj0 + lead)
            jmax = max(jmax, jj0 + 3 - trail)
            kts.append((kt, lead, trail, cls4))
        plan[g] = kts
    if jmin > jmax:
        jmin, jmax = 0, 0
    return plan, jmin, jmax


def _get_compiled(mask_bytes, mask):
    if mask_bytes in _CACHE:
        return _CACHE[mask_bytes]
    block_cls, partials = _classify_mask(mask)
    plan, jmin, jmax = _plan_attn(block_cls)
    n_strip = jmax - jmin + 1
    nca = build_kernel_a(plan, jmin, n_strip, len(partials))
    nca.compile()
    ncb = build_kernel_b()
    ncb.compile()
    _CACHE[mask_bytes] = (nca, ncb, jmin, n_strip, partials)
    return _CACHE[mask_bytes]


def kernel(x, mask, w1, b1, w2, b2, g1, beta1, g2, beta2, pos_w):
    x = np.asarray(x, np.float32)
    w1 = np.asarray(w1, np.float32)
    b1 = np.asarray(b1, np.float32)
    w2 = np.asarray(w2, np.float32)
    b2 = np.asarray(b2, np.float32)
    g1 = np.asarray(g1, np.float32)
    beta1 = np.asarray(beta1, np.float32)
    g2 = np.asarray(g2, np.float32)
    beta2 = np.asarray(beta2, np.float32)
    pos_w = np.asarray(pos_w, np.float32)
    mask_np = np.asarray(mask)

    nca, ncb, jmin, n_strip, partials = _get_compiled(mask_np.tobytes(), mask_np)

    trace = bool(int(os.environ.get("HSTU_TRACE", "0")))
    strip = np.zeros((P, n_strip * P), np.float32)
    pidx = np.arange(P)[:, None]
    fidx = np.arange(P)[None, :]
    for i in range(n_strip):
        jj = jmin + i
        base = M - 1 - P * (jj - 15)
        strip[:, i * P:(i + 1) * P] = pos_w[base + pidx - fidx]
    inv128 = np.full((P, P), 1.0 / P, np.float32)
    parts_arr = (np.stack(partials) if partials
                 else np.zeros((1, P, P), np.float32))

    import ml_dtypes as _mld
    bfh = _mld.bfloat16
    xT = [np.ascontiguousarray(x[b].T) for b in range(B)]
    xTb = [np.ascontiguousarray(t.astype(bfh)) for t in xT]
    in_maps_a = []
    for c in range(8):
        b, hg = divmod(c, 4)
        heads = [4 * hg + i for i in range(4)]
        rows_q = np.concatenate([np.arange(D + h * HD, D + (h + 1) * HD)
                                 for h in heads])
        rows_k = np.concatenate([np.arange(2 * D + h * HD, 2 * D + (h + 1) * HD)
                                 for h in heads])
        rows_u = np.concatenate([np.arange(h * HD, (h + 1) * HD) for h in heads])
        rows_v = np.concatenate([np.arange(3 * D + h * HD, 3 * D + (h + 1) * HD)
                                 for h in heads])
        rows_qku = np.concatenate([rows_q, rows_k, rows_u])
        b1_2d = np.ascontiguousarray(b1[rows_qku].reshape(6, P).T)
        b1v_1 = np.broadcast_to(b1[rows_v][None, :], (P, DC))
        in_maps_a.append(dict(
            xT=xTb[b],
            w1T_qku=np.ascontiguousarray(w1[rows_qku].T.astype(bfh)),
            w1T_v=np.ascontiguousarray(w1[rows_v].T.astype(bfh)),
            b1_2d=b1_2d,
            b1v_bc2=np.ascontiguousarray(
                np.concatenate([b1v_1, b1v_1], axis=1).astype(bfh)),
            inv128=inv128.astype(bfh), strip=strip, parts=parts_arr,
        ))
    res_a = run_bass_kernel_spmd(nca, in_maps_a, core_ids=list(range(8)),
                                 trace=trace)

    import ml_dtypes as _mld2
    yT_full = [np.empty((D, S), _mld2.bfloat16) for _ in range(B)]
    for c in range(8):
        b, hg = divmod(c, 4)
        yT_full[b][hg * DC:(hg + 1) * DC] = res_a.results[c]["yT_out"]

    # phase-2 host precompute: fold ln1's gamma into w2, rank-1 constants
    import ml_dtypes
    bf = ml_dtypes.bfloat16
    w2g = (w2.astype(np.float64) * g1.astype(np.float64)[None, :])
    w2gT = np.ascontiguousarray(w2g.T.astype(bf))
    c1 = w2g.sum(axis=1)
    cb = w2.astype(np.float64) @ beta1.astype(np.float64) + b2
    cvec = np.concatenate([c1, cb]).astype(bf)[None, :]
    cvec = np.ascontiguousarray(cvec)
    g2b2 = np.ascontiguousarray(
        np.concatenate([g2.reshape(8, P).T, beta2.reshape(8, P).T], axis=1))
    onesp = np.full((P, 1), 1.0 / D, np.float32)
    monesp = np.full((P, 1), -1.0 / D, bf)
    monespf = np.full((P, 1), -1.0 / D, np.float32)
    ones512 = np.ones((1, 512), bf)
    ident = np.eye(P, dtype=bf)
    in_maps_b = []
    for c in range(8):
        b, qc = divmod(c, 4)
        sl = slice(qc * 512, (qc + 1) * 512)
        in_maps_b.append(dict(
            yT=np.ascontiguousarray(yT_full[b][:, sl]),
            xTs=np.ascontiguousarray(xT[b][:, sl].astype(bf)),
            w2gT=w2gT, cvec=cvec, g2b2=g2b2,
            onesp=onesp, monesp=monesp, monespf=monespf,
            ones512=ones512, ident=ident,
        ))
    res_b = run_bass_kernel_spmd(ncb, in_maps_b, core_ids=list(range(8)),
                                 trace=trace)

    out = np.empty((B, S, D), np.float32)
    for c in range(8):
        b, qc = divmod(c, 4)
        out[b, qc * 512:(qc + 1) * 512] = res_b.results[c]["outT"].T
    kernel.last_results = (res_a, res_b)
    kernel.last_ncs = (nca, ncb)
    return out


# revision 6
# speedup vs baseline: 1.0293x; 1.0293x over previous
"""HSTU block kernel v2 for 8 TRN2 NeuronCores (Bass/Tile, fp32r matmuls).

Sharding: phase 1 (f1 + attention + u-gating) is data-parallel over batch
(B=2) x tensor-parallel over head groups (4 heads/core = 2 pairs). Phase 2
(ln1 -> f2 -> +residual -> ln2) is row-parallel (512 rows/core), with ln1
folded analytically into the f2 weights so no normalization pass is needed
before the matmul.

Key layout: heads are processed in PAIRS occupying SBUF partitions [0:64] /
[64:128]; QK/AV matmuls for the two heads auto-derive row/col tile_position
(64-row / 64-col groups) and run concurrently in the PE array on HW.
Attention score tiles for a pair live in one 2-bank [128,1024] PSUM tile so
bias-add / mask / silu process both heads in one instruction. The relative
position bias adds run on DVE + GpSimd(Pool), load-balanced; the PE never
touches them.
"""
import os
import numpy as np

import concourse.bacc as bacc
import concourse.mybir as mybir
from concourse import library_config
from concourse.tile import TileContext
from concourse.bass_utils import run_bass_kernel_spmd

fp32 = mybir.dt.float32
fp32r = mybir.dt.float32r
bf16 = mybir.dt.bfloat16
AF = mybir.ActivationFunctionType
ALU = mybir.AluOpType

B, S, D, H, M = 2, 2048, 1024, 16, 4096
HD = D // H          # 64
EPS = 1e-5
P = 128
NB = S // P          # 16 seq blocks of 128
NG = S // 512        # 4 q-groups of 512
DC = 4 * HD          # 256 features per core in phase 1

_CACHE = {}


class Balancer:
    """Greedy least-loaded assignment of elementwise ops to DVE / Pool.

    GPSIMD (Pool) has no PSUM port, so ops touching PSUM are DVE-only.
    """

    def __init__(self, nc, dve_bias=0.0, pool_bias=0.0):
        self.nc = nc
        self.load = {"dve": dve_bias, "pool": pool_bias}

    def pick(self, width, psum=False):
        # cost estimates per engine for a [128, width]-elem op (ns)
        cd = width * 1.0417 + 250
        cp = width * 0.833 / 0.42 + 131
        if psum or self.load["dve"] + cd <= self.load["pool"] + cp:
            self.load["dve"] += cd
            return self.nc.vector
        self.load["pool"] += cp
        return self.nc.gpsimd


# ---------------------------------------------------------------- kernel A
def build_kernel_a(plan, jmin, n_strip, n_partial):
    """plan[g] = [(kt, lead, trail, cls4)], strip holds tiles jmin..jmin+n_strip-1."""
    nc = bacc.Bacc("TRN2", target_bir_lowering=False, debug=False, num_devices=8)

    xT = nc.dram_tensor("xT", [D, S], bf16, kind="ExternalInput")
    w1T_qku = nc.dram_tensor("w1T_qku", [D, 768], bf16, kind="ExternalInput")
    w1T_v = nc.dram_tensor("w1T_v", [D, DC], bf16, kind="ExternalInput")
    b1_2d = nc.dram_tensor("b1_2d", [P, 6], fp32, kind="ExternalInput")
    b1v_bc2 = nc.dram_tensor("b1v_bc2", [P, 512], bf16, kind="ExternalInput")
    inv128 = nc.dram_tensor("inv128", [P, P], bf16, kind="ExternalInput")
    strip = nc.dram_tensor("strip", [P, n_strip * P], fp32, kind="ExternalInput")
    parts = nc.dram_tensor("parts", [max(n_partial, 1), P, P], fp32,
                           kind="ExternalInput")
    yT_out = nc.dram_tensor("yT_out", [DC, S], bf16, kind="ExternalOutput")

    with TileContext(nc) as tc:
        with tc.tile_pool(name="const", bufs=1) as cpool, \
             tc.tile_pool(name="wpool", bufs=2) as wpool, \
             tc.tile_pool(name="big", bufs=1) as big, \
             tc.tile_pool(name="att", bufs=4) as apool, \
             tc.tile_pool(name="out", bufs=3) as opool, \
             tc.tile_pool(name="ps", bufs=1, space="PSUM") as ps:

            bal = Balancer(nc)

            # ---- critical-path DMAs: w1 qku interleaved with slab-0 x ----
            # tiny b1 first (gates the first silu); ACT silu-table prewarm
            b1_sb = cpool.tile([P, 6], fp32, name="b1_sb")
            nc.sync.dma_start(b1_sb[:], b1_2d[:])
            warm = cpool.tile([1, 1], fp32, name="warm")
            nc.gpsimd.memset(warm[:], 0.0)
            warm2 = cpool.tile([1, 1], fp32, name="warm2")
            nc.scalar.activation(warm2[:], warm[:], AF.Silu, scale=1.0)
            wq = []
            xs0 = big.tile([P, 8 * 512], bf16, name="xs0", tag="xs", bufs=2)
            xv = xT.rearrange("(c p) s -> p c s", p=P)
            # priority tranche: q columns (fc 0-1) of every weight chunk plus
            # slab-0 x, so the first two f1 chains start as early as possible
            for k in range(8):
                t = big.tile([P, 768], bf16, name=f"wq{k}", tag=f"wq{k}")
                nc.sync.dma_start(t[:, 0:2 * P],
                                  w1T_qku[k * P:(k + 1) * P, 0:2 * P])
                wq.append(t)
                if k % 2 == 1:
                    c = k // 2
                    nc.sync.dma_start(
                        xs0.rearrange("p (c s) -> p c s", c=8)[:, 2 * c:2 * c + 2,
                                                              0:512],
                        xv[:, 2 * c:2 * c + 2, 0:512])
            for k in range(8):
                nc.sync.dma_start(wq[k][:, 2 * P:768],
                                  w1T_qku[k * P:(k + 1) * P, 2 * P:768])
            inv128_sb = cpool.tile([P, P], bf16, name="inv128_sb")
            nc.sync.dma_start(inv128_sb[:], inv128[:])
            b1v_sb = cpool.tile([P, 512], bf16, name="b1v_sb")
            nc.sync.dma_start(b1v_sb[:], b1v_bc2[:])
            parts_sb = []
            for i in range(n_partial):
                t = cpool.tile([P, P], fp32, name=f"part{i}", tag=f"part{i}")
                nc.sync.dma_start(t[:], parts[i])
                parts_sb.append(t)
            strip_sb = big.tile([P, n_strip * P], fp32, name="strip_sb",
                                tag="strip")
            nc.sync.dma_start(strip_sb[:], strip[:])
            wv = big.tile([P, 8 * DC], bf16, name="wv", tag="wv")
            nc.sync.dma_start(
                wv.rearrange("p (c f) -> p c f", c=8),
                w1T_v.rearrange("(c p) f -> p c f", p=P))

            # persistent k (pairs) and v
            kh = [big.tile([P, S], bf16, name=f"khp{p}", tag=f"khp{p}")
                  for p in range(2)]
            v_sb = big.tile([P, NB * DC], bf16, name="v_sb", tag="v_sb")
            # K=1 zero operands: one start=True matmul initializes a full
            # avp bank so the col-tiled AV pair can pure-accumulate into it
            zrow = cpool.tile([1, 640], bf16, name="zrow")
            nc.gpsimd.memset(zrow[:], 0.0)

            # ---------------- f1: qku + v, feature-major / natural --------
            def f1_slab(sg):
                sl = slice(sg * 512, (sg + 1) * 512)
                if sg == 0:
                    xs = xs0
                else:
                    xs = big.tile([P, 8 * 512], bf16, name="xs", tag="xs",
                                  bufs=2)
                    nc.sync.dma_start(
                        xs.rearrange("p (c s) -> p c s", c=8),
                        xv[:, :, sl])
                qq = [wpool.tile([P, 512], bf16, name="qq", tag=f"qq{p}",
                                 bufs=2) for p in range(2)]
                uu = [wpool.tile([P, 512], fp32, name="uu", tag=f"uu{p}",
                                 bufs=2) for p in range(2)]
                for fc in range(6):
                    pt = ps.tile([P, 512], fp32, name="f1ps", tag="f1", bufs=2)
                    for k in range(8):
                        nc.tensor.matmul(pt[:], wq[k][:, fc * P:(fc + 1) * P],
                                         xs[:, k * 512:(k + 1) * 512],
                                         start=(k == 0), stop=(k == 7))
                    bias = b1_sb[:, fc:fc + 1]
                    if fc < 2:
                        nc.scalar.activation(qq[fc][:], pt[:], AF.Silu,
                                             bias=bias, scale=1.0)
                    elif fc < 4:
                        nc.scalar.activation(kh[fc - 2][:, sl], pt[:], AF.Silu,
                                             bias=bias, scale=1.0)
                    else:
                        nc.scalar.activation(uu[fc - 4][:], pt[:], AF.Silu,
                                             bias=bias, scale=1.0)
                # v: two chains of 2 seq-blocks each (512 tokens -> 4 blocks)
                for scp in range(2):
                    pt = ps.tile([P, 512], fp32, name="fvps", tag="f1", bufs=2)
                    for k in range(8):
                        for sc2 in range(2):
                            tok = (2 * scp + sc2) * P
                            first_mm = (k == 0 and sc2 == 0)
                            nc.tensor.matmul(
                                pt[:, sc2 * DC:(sc2 + 1) * DC],
                                xs[:, k * 512 + tok:k * 512 + tok + P],
                                wv[:, k * DC:(k + 1) * DC],
                                start=first_mm, stop=False)
                    nc.tensor.matmul(pt[:], inv128_sb[:], b1v_sb[:],
                                     start=False, stop=True)
                    base = (4 * sg + 2 * scp) * DC
                    nc.scalar.activation(v_sb[:, base:base + 512], pt[:],
                                         AF.Silu, scale=1.0)
                return qq, uu

            # ---------------- attention (per head pair) ----------------
            def attn_pair(p, g, qq, uu):
                kts = plan[g]
                if not kts:
                    return
                last_kt = kts[-1][0]
                full_first = (kts[0][1] == 0 and kts[0][2] == 0)
                avps = []
                for hp in range(2):
                    t = ps.tile([64, 512], fp32, name="avp", tag="avp", bufs=2)
                    if not full_first:
                        nc.tensor.matmul(t[:], zrow[:, 0:64],
                                         zrow[:, P:P + 512],
                                         start=True, stop=False)
                    avps.append(t)
                for ki, (kt, lead, trail, cls4) in enumerate(kts):
                    off, end = lead * P, 512 - trail * P
                    w = end - off
                    spp = ps.tile([P, 1024], fp32, name="spp", tag="spp",
                                  bufs=2)
                    for hp in range(2):
                        hs = slice(hp * 64, hp * 64 + 64)
                        nc.tensor.matmul(
                            spp[:, hp * 512 + off:hp * 512 + end],
                            kh[p][hs, kt * P:(kt + 1) * P],
                            qq[p][hs, off:end],
                            start=True, stop=True)
                    sb0 = (4 * g - kt + 15 - jmin) * P
                    att_in = apool.tile([P, 1024], fp32, name="att_in",
                                        tag="att_in", bufs=3)
                    sv = spp.rearrange("a (two w) -> a two w", two=2)
                    ai = att_in.rearrange("a (two w) -> a two w", two=2)
                    st = strip_sb[:, sb0 + off:sb0 + end].unsqueeze(1) \
                        .broadcast_to((P, 2, w))
                    nc.vector.tensor_tensor(ai[:, :, off:end],
                                            sv[:, :, off:end], st, ALU.add)
                    for j in range(lead, 4 - trail):
                        c = cls4[j]
                        if c >= 2:
                            aj = ai[:, :, j * P:(j + 1) * P]
                            pj = parts_sb[c - 2][:].unsqueeze(1) \
                                .broadcast_to((P, 2, P))
                            nc.gpsimd.tensor_tensor(aj, aj, pj, ALU.mult)
                    att = apool.tile([P, 1024], bf16, name="att", tag="att",
                                     bufs=4)
                    av = att.rearrange("a (two w) -> a two w", two=2)
                    nc.scalar.activation(av[:, :, off:end], ai[:, :, off:end],
                                         AF.Silu, scale=1.0)
                    vb = kt * DC + p * P
                    for hp in range(2):
                        nc.tensor.matmul(
                            avps[hp][:, off:end],
                            v_sb[:, vb + hp * 64:vb + hp * 64 + 64],
                            att[:, hp * 512 + off:hp * 512 + end],
                            start=(full_first and ki == 0),
                            stop=(kt == last_kt))
                yg = opool.tile([P, 512], bf16, name="yg", tag="yg", bufs=2)
                for hp in range(2):
                    nc.vector.tensor_tensor(yg[hp * 64:hp * 64 + 64, :],
                                            avps[hp][:],
                                            uu[p][hp * 64:hp * 64 + 64, :],
                                            ALU.mult)
                nc.sync.dma_start(
                    yT_out[p * P:(p + 1) * P, g * 512:(g + 1) * 512], yg[:])

            for sg in range(NG):
                qq, uu = f1_slab(sg)
                attn_pair(0, sg, qq, uu)
                attn_pair(1, sg, qq, uu)
    return nc


# ---------------------------------------------------------------- kernel B
def build_kernel_b():
    nc = bacc.Bacc("TRN2", target_bir_lowering=False, debug=False, num_devices=8)

    yT = nc.dram_tensor("yT", [D, 512], bf16, kind="ExternalInput")
    xTs = nc.dram_tensor("xTs", [D, 512], bf16, kind="ExternalInput")
    w2gT = nc.dram_tensor("w2gT", [D, D], bf16, kind="ExternalInput")
    cvec = nc.dram_tensor("cvec", [1, 2 * D], bf16, kind="ExternalInput")
    g2b2 = nc.dram_tensor("g2b2", [P, 16], fp32, kind="ExternalInput")
    onesp = nc.dram_tensor("onesp", [P, 1], fp32, kind="ExternalInput")
    monesp = nc.dram_tensor("monesp", [P, 1], bf16, kind="ExternalInput")
    monespf = nc.dram_tensor("monespf", [P, 1], fp32, kind="ExternalInput")
    ones512 = nc.dram_tensor("ones512", [1, 512], bf16, kind="ExternalInput")
    ident = nc.dram_tensor("ident", [P, P], bf16, kind="ExternalInput")
    outT = nc.dram_tensor("outT", [D, 512], bf16, kind="ExternalOutput")

    yv = yT.rearrange("(c p) s -> p c s", p=P)
    xv = xTs.rearrange("(c p) s -> p c s", p=P)
    wv = w2gT.rearrange("(c p) f -> p c f", p=P)

    with TileContext(nc) as tc:
        with tc.tile_pool(name="const", bufs=1) as cpool, \
             tc.tile_pool(name="big", bufs=1) as big, \
             tc.tile_pool(name="tmp", bufs=3) as tp, \
             tc.tile_pool(name="ps", bufs=1, space="PSUM") as ps:

            bal = Balancer(nc)

            nc.gpsimd.load_library(library_config.proxy)
            eps_sb = cpool.tile([1, 1], fp32, name="eps_sb")
            nc.gpsimd.memset(eps_sb[:], EPS)
            warm = cpool.tile([1, 1], fp32, name="warm")
            nc.scalar.activation(warm[:], eps_sb[:], AF.Sqrt, scale=1.0)

            # DMA order tuned for the dependency graph: first weight chunk
            # (starts f2 chain 0), tiny consts, y (stats + f2 rhs), xts +
            # cvec (gate the rank-1 chains), then remaining weight chunks
            # which pace the later f2 accumulation chains.
            yt = big.tile([P, 8 * 512], bf16, name="yt", tag="yt")
            w2g = big.tile([P, 8 * D], bf16, name="w2g", tag="w2g")
            xts = big.tile([P, 8 * 512], bf16, name="xts", tag="xts")
            onesp_sb = cpool.tile([P, 1], fp32r, name="onesp_sb")
            nc.sync.dma_start(onesp_sb[:], onesp[:].bitcast(fp32r))
            monesp_sb = cpool.tile([P, 1], bf16, name="monesp_sb")
            nc.sync.dma_start(monesp_sb[:], monesp[:])
            monespf_sb = cpool.tile([P, 1], fp32r, name="monespf_sb")
            nc.sync.dma_start(monespf_sb[:], monespf[:].bitcast(fp32r))
            for c in range(4):
                nc.sync.dma_start(
                    yt.rearrange("p (c s) -> p c s", c=8)
                    [:, 2 * c:2 * c + 2, :],
                    yv[:, 2 * c:2 * c + 2, :])
            ones512_sb = cpool.tile([1, 512], bf16, name="ones512_sb")
            nc.sync.dma_start(ones512_sb[:], ones512[:])
            cvec_sb = cpool.tile([1, 2 * D], bf16, name="cvec_sb")
            nc.sync.dma_start(cvec_sb[:], cvec[:])
            g2b2_sb = cpool.tile([P, 16], fp32, name="g2b2_sb")
            nc.sync.dma_start(g2b2_sb[:], g2b2[:])
            ident_sb = cpool.tile([P, P], bf16, name="ident_sb")
            nc.sync.dma_start(ident_sb[:], ident[:])
            xvv = xts.rearrange("p (c s) -> p c s", c=8)
            # column tranches: fc 0-3 slices of every k-chunk first, so the
            # first four f2 chains (and their ln2 pipeline work) start while
            # the fc 4-7 slices are still streaming in
            for k in range(8):
                nc.sync.dma_start(
                    w2g[:, k * D:k * D + 512],
                    wv[:, k, 0:512])
                if k < 4:
                    nc.sync.dma_start(xvv[:, 2 * k:2 * k + 2, :],
                                      xv[:, 2 * k:2 * k + 2, :])
            for k in range(8):
                nc.sync.dma_start(
                    w2g[:, k * D + 512:(k + 1) * D],
                    wv[:, k, 512:D])

            def stats(src_slices, tag, mones):
                """accumulate -mean and +mean-of-squares; [1,512] PSUM."""
                n = len(src_slices)
                ss = ps.tile([1, 512], fp32, name=f"ss{tag}", tag="st", bufs=2)
                sq = ps.tile([1, 512], fp32, name=f"sq{tag}", tag="st", bufs=2)
                for k, ch in enumerate(src_slices):
                    nc.tensor.matmul(ss[:], mones[:], ch,
                                     start=(k == 0), stop=(k == n - 1))
                    sqt = tp.tile([P, 512], fp32r, name="sqt", tag="sqt",
                                  bufs=2)
                    nc.scalar.activation(sqt[:], ch, AF.Square,
                                         scale=1.0)
                    nc.tensor.matmul(sq[:], onesp_sb[:], sqt[:],
                                     start=(k == 0), stop=(k == n - 1))
                return ss, sq

            def norm_coefs(nmu, ms, tag):
                """nmu=-mean, ms=E[x^2] -> (rstd, s=-mu*rstd) [1,512] fp32r."""
                m2 = tp.tile([1, 512], fp32, name=f"m2{tag}", tag="vec", bufs=4)
                nc.scalar.activation(m2[:], nmu[:], AF.Square, scale=1.0)
                var = tp.tile([1, 512], fp32, name=f"var{tag}", tag="vec",
                              bufs=4)
                nc.vector.tensor_tensor(var[:], ms[:], m2[:], ALU.subtract)
                sd = tp.tile([1, 512], fp32, name=f"sd{tag}", tag="vec", bufs=4)
                nc.scalar.activation(sd[:], var[:], AF.Sqrt, bias=eps_sb[:],
                                     scale=1.0)
                rstd = tp.tile([1, 512], fp32r, name=f"rstd{tag}", tag="vecr",
                               bufs=4)
                nc.vector.reciprocal(rstd[:].bitcast(fp32), sd[:])
                s = tp.tile([1, 512], bf16, name=f"s{tag}", tag="vecr",
                             bufs=4)
                nc.vector.tensor_tensor(s[:], nmu[:],
                                        rstd[:].bitcast(fp32), ALU.mult)
                return rstd, s

            # ---- ln1 (folded): stats on raw y run concurrently with f2 ----
            ytc = [yt[:, k * 512:(k + 1) * 512] for k in range(8)]
            ss1, sq1 = stats(ytc, "a", monesp_sb)
            rstd1, s1 = norm_coefs(ss1, sq1, "a")
            pa_sb = tp.tile([P, 512], fp32, name="pa_sb", tag="bcs", bufs=3)
            nc.gpsimd.partition_broadcast(pa_sb[:], rstd1[:].bitcast(fp32))

            # ---- f2 on RAW y; post-scale z; rank-1 + residual; ln2 ----
            # processed in token halves so ln2 stats + apply of half 0
            # overlap the f2 accumulation chains of half 1
            y2 = big.tile([P, 8 * 512], bf16, name="y2", tag="y2")
            outl = big.tile([P, 8 * 512], bf16, name="outl", tag="outl")
            ov = outT.rearrange("(c p) s -> p c s", p=P)
            W = 256

            def norm_coefs_h(nmu, ms, tag):
                m2 = tp.tile([1, W], fp32, name=f"m2{tag}", tag="vech", bufs=4)
                nc.scalar.activation(m2[:], nmu[:], AF.Square, scale=1.0)
                var = tp.tile([1, W], fp32, name=f"var{tag}", tag="vech",
                              bufs=4)
                nc.vector.tensor_tensor(var[:], ms[:], m2[:], ALU.subtract)
                sd = tp.tile([1, W], fp32, name=f"sd{tag}", tag="vech", bufs=4)
                nc.scalar.activation(sd[:], var[:], AF.Sqrt, bias=eps_sb[:],
                                     scale=1.0)
                rstd = tp.tile([1, W], bf16, name=f"rstd{tag}", tag="vech",
                               bufs=4)
                sh = tp.tile([1, W], bf16, name=f"sh{tag}", tag="vech", bufs=4)
                with nc.allow_low_precision(reason="bf16 ln2 coefs, 2e-2 tol"):
                    nc.vector.reciprocal(rstd[:], sd[:])
                    nc.vector.tensor_tensor(sh[:], nmu[:], rstd[:], ALU.mult)
                return rstd, sh

            # prebuild rank-1+residual tiles in SBUF (PE+ACT, overlaps
            # the w2g DMA window) so the endgame add can run on Pool
            rr_sb = {}
            for h in range(2):
                t0 = h * W
                for fc in range(8):
                    rr = ps.tile([P, W], fp32, name=f"rr{fc}", tag="rr",
                                 bufs=2)
                    nc.tensor.matmul(rr[:], ident_sb[:],
                                     xts[:, fc * 512 + t0:fc * 512 + t0 + W],
                                     start=True, stop=False)
                    nc.tensor.matmul(rr[:],
                                     cvec_sb[:, D + fc * P:D + (fc + 1) * P],
                                     ones512_sb[:, 0:W],
                                     start=False, stop=False)
                    nc.tensor.matmul(rr[:], cvec_sb[:, fc * P:(fc + 1) * P],
                                     s1[:, t0:t0 + W], start=False, stop=True)
                    t = big.tile([P, W], fp32, name=f"rrs{h}_{fc}",
                                 tag=f"rrs{h}_{fc}")
                    nc.scalar.activation(t[:], rr[:], AF.Copy, scale=1.0)
                    rr_sb[(h, fc)] = t

            for h in range(2):
                t0 = h * W
                ss2 = ps.tile([1, W], fp32, name=f"ss2{h}", tag="st", bufs=2)
                sq2 = ps.tile([1, W], fp32, name=f"sq2{h}", tag="st", bufs=2)
                for fc in range(8):
                    pt = ps.tile([P, W], fp32, name=f"f2ps{fc}", tag="f2",
                                 bufs=4)
                    for k in range(8):
                        nc.tensor.matmul(
                            pt[:],
                            w2g[:, k * D + fc * P:k * D + (fc + 1) * P],
                            yt[:, k * 512 + t0:k * 512 + t0 + W],
                            start=(k == 0), stop=(k == 7))
                    ch = y2[:, fc * 512 + t0:fc * 512 + t0 + W]
                    nc.vector.tensor_tensor(ch, pt[:],
                                            pa_sb[:, t0:t0 + W], ALU.mult)
                    nc.gpsimd.tensor_tensor(ch, ch, rr_sb[(h, fc)][:],
                                            ALU.add)
                    nc.tensor.matmul(ss2[:], monesp_sb[:], ch,
                                     start=(fc == 0), stop=(fc == 7))
                    sqt = tp.tile([P, W], fp32r, name="sqt2", tag="sqt2",
                                  bufs=2)
                    nc.scalar.activation(sqt[:], ch, AF.Square,
                                         scale=1.0)
                    nc.tensor.matmul(sq2[:], onesp_sb[:], sqt[:],
                                     start=(fc == 0), stop=(fc == 7))

                rstd2, s2 = norm_coefs_h(ss2, sq2, f"b{h}")
                pa2_sb = tp.tile([P, W], bf16, name="pa2_sb", tag="bcs",
                                 bufs=3)
                nc.gpsimd.partition_broadcast(pa2_sb[:], rstd2[:])
                pb2_sb = tp.tile([P, W], bf16, name="pb2_sb", tag="bcs",
                                 bufs=3)
                nc.gpsimd.partition_broadcast(pb2_sb[:], s2[:])
                for fc in range(8):
                    yc = y2[:, fc * 512 + t0:fc * 512 + t0 + W]
                    u1 = tp.tile([P, W], bf16, name="u1", tag="u1", bufs=4)
                    with nc.allow_low_precision(reason="bf16 ln2 apply, 2e-2 tol"):
                        eng = bal.pick(W)
                        eng.tensor_tensor(u1[:], yc, pa2_sb[:], ALU.mult)
                        eng = bal.pick(W)
                        eng.tensor_tensor(u1[:], u1[:], pb2_sb[:], ALU.add)
                        oc = outl[:, fc * 512 + t0:fc * 512 + t0 + W]
                        eng = bal.pick(W)
                        eng.tensor_scalar(oc, u1[:],
                                          g2b2_sb[:, fc:fc + 1],
                                          g2b2_sb[:, 8 + fc:8 + fc + 1],
                                          ALU.mult, ALU.add)
                    if fc % 4 == 3:
                        nc.sync.dma_start(
                            ov[:, fc - 3:fc + 1, t0:t0 + W],
                            outl.rearrange("p (c s) -> p c s", c=8)
                            [:, fc - 3:fc + 1, t0:t0 + W])
    return nc


# ---------------------------------------------------------------- host side
def _classify_mask(mask):
    keep = (mask.reshape(S, S) >= 0)
    block_cls = [[0] * NB for _ in range(NB)]  # [kt][qb]
    partials = []
    pmap = {}
    for kt in range(NB):
        for qb in range(NB):
            sub = keep[qb * P:(qb + 1) * P, kt * P:(kt + 1) * P]
            if sub.all():
                block_cls[kt][qb] = 1
            elif not sub.any():
                block_cls[kt][qb] = 0
            else:
                tile = np.ascontiguousarray(sub.T.astype(np.float32))
                key = tile.tobytes()
                if key not in pmap:
                    pmap[key] = len(partials)
                    partials.append(tile)
                block_cls[kt][qb] = 2 + pmap[key]
    return block_cls, partials


def _plan_attn(block_cls):
    """Per (g, kt): lead/trail skip blocks + strip tile range used."""
    plan = {}
    jmin, jmax = 31, 0
    for g in range(NG):
        kts = []
        for kt in range(NB):
            cls4 = [block_cls[kt][4 * g + j] for j in range(4)]
            if all(c == 0 for c in cls4):
                continue
            lead = 0
            while cls4[lead] == 0:
                lead += 1
            trail = 0
            while cls4[3 - trail] == 0:
                trail += 1
            jj0 = 4 * g - kt + 15
            jmin = min(jmin, jj0 + lead)
            jmax = max(jmax, jj0 + 3 - trail)
            kts.append((kt, lead, trail, cls4))
        plan[g] = kts
    if jmin > jmax:
        jmin, jmax = 0, 0
    return plan, jmin, jmax


def _get_compiled(mask_bytes, mask):
    if mask_bytes in _CACHE:
        return _CACHE[mask_bytes]
    block_cls, partials = _classify_mask(mask)
    plan, jmin, jmax = _plan_attn(block_cls)
    n_strip = jmax - jmin + 1
    nca = build_kernel_a(plan, jmin, n_strip, len(partials))
    nca.compile()
    ncb = build_kernel_b()
    ncb.compile()
    _CACHE[mask_bytes] = (nca, ncb, jmin, n_strip, partials)
    return _CACHE[mask_bytes]


def kernel(x, mask, w1, b1, w2, b2, g1, beta1, g2, beta2, pos_w):
    x = np.asarray(x, np.float32)
    w1 = np.asarray(w1, np.float32)
    b1 = np.asarray(b1, np.float32)
    w2 = np.asarray(w2, np.float32)
    b2 = np.asarray(b2, np.float32)
    g1 = np.asarray(g1, np.float32)
    beta1 = np.asarray(beta1, np.float32)
    g2 = np.asarray(g2, np.float32)
    beta2 = np.asarray(beta2, np.float32)
    pos_w = np.asarray(pos_w, np.float32)
    mask_np = np.asarray(mask)

    nca, ncb, jmin, n_strip, partials = _get_compiled(mask_np.tobytes(), mask_np)

    trace = bool(int(os.environ.get("HSTU_TRACE", "0")))
    strip = np.zeros((P, n_strip * P), np.float32)
    pidx = np.arange(P)[:, None]
    fidx = np.arange(P)[None, :]
    for i in range(n_strip):
        jj = jmin + i
        base = M - 1 - P * (jj - 15)
        strip[:, i * P:(i + 1) * P] = pos_w[base + pidx - fidx]
    inv128 = np.full((P, P), 1.0 / P, np.float32)
    parts_arr = (np.stack(partials) if partials
                 else np.zeros((1, P, P), np.float32))

    import ml_dtypes as _mld
    bfh = _mld.bfloat16
    xT = [np.ascontiguousarray(x[b].T) for b in range(B)]
    xTb = [np.ascontiguousarray(t.astype(bfh)) for t in xT]
    in_maps_a = []
    for c in range(8):
        b, hg = divmod(c, 4)
        heads = [4 * hg + i for i in range(4)]
        rows_q = np.concatenate([np.arange(D + h * HD, D + (h + 1) * HD)
                                 for h in heads])
        rows_k = np.concatenate([np.arange(2 * D + h * HD, 2 * D + (h + 1) * HD)
                                 for h in heads])
        rows_u = np.concatenate([np.arange(h * HD, (h + 1) * HD) for h in heads])
        rows_v = np.concatenate([np.arange(3 * D + h * HD, 3 * D + (h + 1) * HD)
                                 for h in heads])
        rows_qku = np.concatenate([rows_q, rows_k, rows_u])
        b1_2d = np.ascontiguousarray(b1[rows_qku].reshape(6, P).T)
        b1v_1 = np.broadcast_to(b1[rows_v][None, :], (P, DC))
        in_maps_a.append(dict(
            xT=xTb[b],
            w1T_qku=np.ascontiguousarray(w1[rows_qku].T.astype(bfh)),
            w1T_v=np.ascontiguousarray(w1[rows_v].T.astype(bfh)),
            b1_2d=b1_2d,
            b1v_bc2=np.ascontiguousarray(
                np.concatenate([b1v_1, b1v_1], axis=1).astype(bfh)),
            inv128=inv128.astype(bfh), strip=strip, parts=parts_arr,
        ))
    res_a = run_bass_kernel_spmd(nca, in_maps_a, core_ids=list(range(8)),
                                 trace=trace)

    import ml_dtypes as _mld2
    yT_full = [np.empty((D, S), _mld2.bfloat16) for _ in range(B)]
    for c in range(8):
        b, hg = divmod(c, 4)
        yT_full[b][hg * DC:(hg + 1) * DC] = res_a.results[c]["yT_out"]

    # phase-2 host precompute: fold ln1's gamma into w2, rank-1 constants
    import ml_dtypes
    bf = ml_dtypes.bfloat16
    w2g = (w2.astype(np.float64) * g1.astype(np.float64)[None, :])
    w2gT = np.ascontiguousarray(w2g.T.astype(bf))
    c1 = w2g.sum(axis=1)
    cb = w2.astype(np.float64) @ beta1.astype(np.float64) + b2
    cvec = np.concatenate([c1, cb]).astype(bf)[None, :]
    cvec = np.ascontiguousarray(cvec)
    g2b2 = np.ascontiguousarray(
        np.concatenate([g2.reshape(8, P).T, beta2.reshape(8, P).T], axis=1))
    onesp = np.full((P, 1), 1.0 / D, np.float32)
    monesp = np.full((P, 1), -1.0 / D, bf)
    monespf = np.full((P, 1), -1.0 / D, np.float32)
    ones512 = np.ones((1, 512), bf)
    ident = np.eye(P, dtype=bf)
    in_maps_b = []
    for c in range(8):
        b, qc = divmod(c, 4)
        sl = slice(qc * 512, (qc + 1) * 512)
        in_maps_b.append(dict(
            yT=np.ascontiguousarray(yT_full[b][:, sl]),
            xTs=np.ascontiguousarray(xT[b][:, sl].astype(bf)),
            w2gT=w2gT, cvec=cvec, g2b2=g2b2,
            onesp=onesp, monesp=monesp, monespf=monespf,
            ones512=ones512, ident=ident,
        ))
    res_b = run_bass_kernel_spmd(ncb, in_maps_b, core_ids=list(range(8)),
                                 trace=trace)

    out = np.empty((B, S, D), np.float32)
    for c in range(8):
        b, qc = divmod(c, 4)
        out[b, qc * 512:(qc + 1) * 512] = res_b.results[c]["outT"].T
    kernel.last_results = (res_a, res_b)
    kernel.last_ncs = (nca, ncb)
    return out
